# revision 1
# baseline (speedup 1.0000x reference)
"""Trainium2 Bass kernel for nn_GatheringLoss (retrieval_knn).

Reference computation:
    q = queries.reshape(-1, C)              # [R, C], R = N*L = 65536
    score = q @ items.T                     # [R, M]
    idx = argmax(softmax(score), axis=1)    # == argmax(score) (softmax monotonic)
    loss = mean((q - items[idx])**2)

Algebraic restructuring (avoids the gather entirely):
    ||q_r - x_{idx_r}||^2 = ||q_r||^2 - 2*smax_r + ||x_{idx_r}||^2
    loss = (sum_r ||q_r||^2 - 2*sum_r smax_r + sum_r ||x_{idx_r}||^2) / (R*C)

Per-row quantities produced on device:
  - smax_r = max_m score[r, m]           (plain fp32 max-reduce from PSUM)
  - nsum_r = sum_m (score[r, m] >= smax_r) * ||x_m||^2
             (fused scalar_tensor_tensor: indicator-weighted sum = the
              matched item's exact squared norm; fp32 ties are ~never)
  - sum ||q||^2 accumulated per c-channel via ScalarE Square+accum.

Sharding: data-parallel over the flattened row axis, 8192 rows/core on 8
cores; items table replicated. The matmul runs in bf16 (PE native rate) with
fp32 PSUM accumulation; norms are exact fp32.

Host side only reshapes/casts inputs, and sums 3 partial sums per partition
per core (the "all-reduce" of the scalar mean).
"""

import numpy as np
import ml_dtypes

# Problem constants (hardcoded per the task contract).
N, L, C, M = 64, 1024, 512, 2048
ROWS = N * L                  # 65536
NCORES = 8
RPC = ROWS // NCORES          # 8192 rows per core
P = 128                       # partitions / row-block size
KC = C // P                   # 4 contraction chunks of 128
NJ = M // 512                 # 4 item chunks of 512 (one PSUM bank each)

_CACHE = {}

# "hist": DVE max + 2x is_ge mask; PE counts matmuls; host norms-dot (fast).
# "stt": DVE max + fused indicator*norm sum (simpler, slower: 2 fp32 passes).
KERNEL_VARIANT = "hist"


def _build_hist(n_rowblocks, num_devices, repeat=1):
    """Histogram variant.

    Per row-block: matmul scores into PSUM (two 2-bank halves), ScalarE
    copies them to SBUF fp32, DVE takes the row max (1x) and an is_ge
    mask at 2x (single-src SBUF fp32), and PE folds the mask over rows
    (ones.T @ mask) into 4 persistent PSUM count banks. The matched item
    norm sum becomes the host-side dot  sum_m counts[m] * ||x_m||^2.
    """
    import concourse.mybir as mybir
    import concourse.tile as tile
    from concourse import bacc
    from contextlib import ExitStack

    nc = bacc.Bacc(
        "TRN2", target_bir_lowering=False, debug=False, num_devices=num_devices
    )
    bf16 = mybir.dt.bfloat16
    f32 = mybir.dt.float32

    qt_d = nc.dram_tensor("qt", [n_rowblocks, P, KC, P], bf16, kind="ExternalInput")
    it_d = nc.dram_tensor("it", [KC, P, M], bf16, kind="ExternalInput")
    out_d = nc.dram_tensor("out3", [P, 4], f32, kind="ExternalOutput")
    cnt_d = nc.dram_tensor("cnt", [1, M], f32, kind="ExternalOutput")

    with ExitStack() as ctx:
        tc = ctx.enter_context(tile.TileContext(nc))
        singles = ctx.enter_context(tc.tile_pool(name="singles", bufs=1))
        qpool = ctx.enter_context(tc.tile_pool(name="qpool", bufs=4))
        scpool = ctx.enter_context(tc.tile_pool(name="scpool", bufs=2))
        mkpool = ctx.enter_context(tc.tile_pool(name="mkpool", bufs=2))
        sqpool = ctx.enter_context(tc.tile_pool(name="sqpool", bufs=2))
        psum = ctx.enter_context(tc.tile_pool(name="psum", bufs=2, space="PSUM"))
        cntp = ctx.enter_context(tc.tile_pool(name="cntp", bufs=1, space="PSUM"))
        accp = ctx.enter_context(tc.tile_pool(name="accp", bufs=1))

        items_sb = []
        for kc in range(KC):
            t_ = singles.tile([P, M], bf16, name=f"items{kc}")
            nc.sync.dma_start(out=t_, in_=it_d.ap()[kc])
            items_sb.append(t_)
        ones_sb = singles.tile([P, 1], bf16, name="ones_sb")
        nc.vector.memset(ones_sb, 1.0)

        m_all = accp.tile([P, n_rowblocks], f32, name="m_all")
        q2_all = accp.tile([P, n_rowblocks], f32, name="q2_all")
        cnt_ps = [cntp.tile([1, 512], f32, name=f"cnt{j}") for j in range(NJ)]

        for rep in range(repeat):
         for rb in range(n_rowblocks):
            qt_t = qpool.tile([P, KC, P], bf16, name="qt_t")
            nc.sync.dma_start(out=qt_t, in_=qt_d.ap()[rb])

            score_sb = scpool.tile([P, M], f32, name="score_sb")
            for h in range(2):
                sps = psum.tile([P, 1024], f32, name="sps")
                for kc in range(KC):
                    for j in range(2):
                        nc.tensor.matmul(
                            sps[:, j * 512:(j + 1) * 512],
                            lhsT=qt_t[:, kc, :],
                            rhs=items_sb[kc][:, h * 1024 + j * 512:
                                             h * 1024 + (j + 1) * 512],
                            start=(kc == 0),
                            stop=(kc == KC - 1),
                        )
                nc.scalar.copy(score_sb[:, h * 1024:(h + 1) * 1024], sps)

            nc.vector.tensor_reduce(
                m_all[:, rb:rb + 1], score_sb,
                axis=mybir.AxisListType.X, op=mybir.AluOpType.max,
            )
            mask = mkpool.tile([P, M], bf16, name="mask")
            nc.vector.tensor_scalar(
                out=mask, in0=score_sb,
                scalar1=m_all[:, rb:rb + 1], scalar2=None,
                op0=mybir.AluOpType.is_ge,
            )
            for j in range(NJ):
                nc.tensor.matmul(
                    cnt_ps[j][0:1, :],
                    lhsT=ones_sb[:, 0:1],
                    rhs=mask[:, j * 512:(j + 1) * 512],
                    start=(rep == 0 and rb == 0),
                    stop=(rep == repeat - 1 and rb == n_rowblocks - 1),
                )

            sq = sqpool.tile([P, KC, P], bf16, name="sq")
            nc.scalar.activation(
                out=sq, in_=qt_t,
                func=mybir.ActivationFunctionType.Square,
                accum_out=q2_all[:, rb:rb + 1],
            )

        outs = accp.tile([P, 4], f32, name="outs")
        nc.vector.tensor_reduce(
            outs[:, 0:1], q2_all, axis=mybir.AxisListType.X, op=mybir.AluOpType.add
        )
        nc.vector.tensor_reduce(
            outs[:, 1:2], m_all, axis=mybir.AxisListType.X, op=mybir.AluOpType.add
        )
        nc.vector.memset(outs[:, 2:4], 0.0)
        nc.sync.dma_start(out=out_d.ap(), in_=outs)

        cnt_sb = accp.tile([1, M], f32, name="cnt_sb")
        for j in range(NJ):
            nc.scalar.copy(cnt_sb[0:1, j * 512:(j + 1) * 512], cnt_ps[j][0:1, :])
        nc.sync.dma_start(out=cnt_d.ap(), in_=cnt_sb)

    nc.compile()
    return nc


def _build(n_rowblocks, num_devices, repeat=1):
    """Build the Bass module (one NEFF, run SPMD on all cores).

    repeat > 1 re-runs the whole inner loop (same data, overwriting the
    accumulators) — used only for slope-based HW timing in bench.py.
    """
    import concourse.mybir as mybir
    import concourse.tile as tile
    from concourse import bacc
    from contextlib import ExitStack

    nc = bacc.Bacc(
        "TRN2",
        target_bir_lowering=False,
        debug=False,
        num_devices=num_devices,
    )

    bf16 = mybir.dt.bfloat16
    f32 = mybir.dt.float32

    # qt[rb, c, kc, row] = q[rb*128 + row, kc*128 + c]  (pre-transposed on host)
    qt_d = nc.dram_tensor("qt", [n_rowblocks, P, KC, P], bf16, kind="ExternalInput")
    # it[kc, c, m] = items[m, kc*128 + c]
    it_d = nc.dram_tensor("it", [KC, P, M], bf16, kind="ExternalInput")
    # nb[p, m] = ||items[m]||^2  (replicated across partitions)
    nb_d = nc.dram_tensor("nb", [P, M], f32, kind="ExternalInput")
    # out3[p, 0..2] = (sum q^2, sum smax, sum norm_at_argmax) per partition
    out_d = nc.dram_tensor("out3", [P, 4], f32, kind="ExternalOutput")

    with ExitStack() as ctx:
        tc = ctx.enter_context(tile.TileContext(nc))
        singles = ctx.enter_context(tc.tile_pool(name="singles", bufs=1))
        qpool = ctx.enter_context(tc.tile_pool(name="qpool", bufs=4))
        spool = ctx.enter_context(tc.tile_pool(name="spool", bufs=2))
        sqpool = ctx.enter_context(tc.tile_pool(name="sqpool", bufs=2))
        psum = ctx.enter_context(tc.tile_pool(name="psum", bufs=2, space="PSUM"))
        accp = ctx.enter_context(tc.tile_pool(name="accp", bufs=1))

        # Resident tables: one items tile per contraction chunk so the first
        # matmul only waits on the first 512 KB DMA, and the norm table.
        items_sb = []
        for kc in range(KC):
            t_ = singles.tile([P, M], bf16, name=f"items{kc}")
            nc.sync.dma_start(out=t_, in_=it_d.ap()[kc])
            items_sb.append(t_)
        nb_sb = singles.tile([P, M], f32, name="nbsb")
        nc.sync.dma_start(out=nb_sb, in_=nb_d.ap())

        m_all = accp.tile([P, n_rowblocks], f32, name="m_all")
        t_all = accp.tile([P, n_rowblocks], f32, name="t_all")
        q2_all = accp.tile([P, n_rowblocks], f32, name="q2_all")

        for rep in range(repeat):
         for rb in range(n_rowblocks):
            qt_t = qpool.tile([P, KC, P], bf16, name="qt_t")
            nc.sync.dma_start(out=qt_t, in_=qt_d.ap()[rb])

            score = psum.tile([P, M], f32, name="score")
            for kc in range(KC):
                for j in range(NJ):
                    nc.tensor.matmul(
                        score[:, j * 512:(j + 1) * 512],
                        lhsT=qt_t[:, kc, :],
                        rhs=items_sb[kc][:, j * 512:(j + 1) * 512],
                        start=(kc == 0),
                        stop=(kc == KC - 1),
                    )

            # Pass 1: exact fp32 row max.
            nc.vector.tensor_reduce(
                m_all[:, rb:rb + 1],
                score[:, :],
                axis=mybir.AxisListType.X,
                op=mybir.AluOpType.max,
            )
            # Pass 2: fused (score >= max) * norm -> sum = norm at argmax.
            scratch = spool.tile([P, M], bf16, name="scratch")
            nc.vector.scalar_tensor_tensor(
                out=scratch,
                in0=score[:, :],
                scalar=m_all[:, rb:rb + 1],
                in1=nb_sb,
                op0=mybir.AluOpType.is_ge,
                op1=mybir.AluOpType.mult,
                accum_out=t_all[:, rb:rb + 1],
            )
            # sum over this row-block of q^2 per c-channel (ScalarE).
            sq = sqpool.tile([P, KC, P], bf16, name="sq")
            nc.scalar.activation(
                out=sq,
                in_=qt_t,
                func=mybir.ActivationFunctionType.Square,
                accum_out=q2_all[:, rb:rb + 1],
            )

        outs = accp.tile([P, 4], f32, name="outs")
        nc.vector.tensor_reduce(
            outs[:, 0:1], q2_all, axis=mybir.AxisListType.X, op=mybir.AluOpType.add
        )
        nc.vector.tensor_reduce(
            outs[:, 1:2], m_all, axis=mybir.AxisListType.X, op=mybir.AluOpType.add
        )
        nc.vector.tensor_reduce(
            outs[:, 2:3], t_all, axis=mybir.AxisListType.X, op=mybir.AluOpType.add
        )
        nc.vector.memset(outs[:, 3:4], 0.0)
        nc.sync.dma_start(out=out_d.ap(), in_=outs)

    nc.compile()
    return nc


def _get_nc(variant=None):
    variant = variant or KERNEL_VARIANT
    key = ("nc", variant, RPC // P, NCORES)
    if key not in _CACHE:
        builder = _build_hist if variant == "hist" else _build
        _CACHE[key] = builder(RPC // P, NCORES)
    return _CACHE[key]


def _prep_core_inputs(queries, items, variant=None):
    """Host-side reshape/cast into per-core input maps."""
    variant = variant or KERNEL_VARIANT
    bf16 = ml_dtypes.bfloat16
    q = np.ascontiguousarray(np.asarray(queries, dtype=np.float32).reshape(ROWS, C))
    items = np.asarray(items, dtype=np.float32)

    qbf = q.astype(bf16)
    # it[kc, c, m]
    itT = np.ascontiguousarray(
        items.astype(bf16).reshape(M, KC, P).transpose(1, 2, 0)
    )
    norms = (items.astype(np.float64) ** 2).sum(axis=1)

    in_maps = []
    nrb = RPC // P
    for r in range(NCORES):
        shard = qbf[r * RPC:(r + 1) * RPC]  # [RPC, C]
        # [rb, row, kc, c] -> [rb, c, kc, row]
        a = np.ascontiguousarray(shard.reshape(nrb, P, KC, P).transpose(0, 3, 2, 1))
        im = {"qt": a, "it": itT}
        if variant != "hist":
            im["nb"] = np.ascontiguousarray(
                np.broadcast_to(norms.astype(np.float32)[None, :], (P, M))
            )
        in_maps.append(im)
    return in_maps, norms


def _assemble_loss(results, norms64=None, variant=None):
    variant = variant or KERNEL_VARIANT
    tot_q2 = 0.0
    tot_m = 0.0
    tot_n = 0.0
    for res in results:
        o = np.asarray(res["out3"], dtype=np.float64)
        tot_q2 += o[:, 0].sum()
        tot_m += o[:, 1].sum()
        if variant == "hist":
            counts = np.asarray(res["cnt"], dtype=np.float64).reshape(M)
            tot_n += float(counts @ norms64)
        else:
            tot_n += o[:, 2].sum()
    loss = (tot_q2 - 2.0 * tot_m + tot_n) / (ROWS * C)
    return np.float32(loss)


def run_on_hw(queries, items, trace=False, trace_kwargs=None):
    """Run on the 8 NeuronCores; returns (loss, BassKernelResults)."""
    from concourse.bass_utils import run_bass_kernel_spmd

    nc = _get_nc()
    in_maps, norms64 = _prep_core_inputs(queries, items)
    try:
        res = run_bass_kernel_spmd(
            nc,
            in_maps,
            core_ids=list(range(NCORES)),
            trace=trace,
            **(trace_kwargs or {}),
        )
    except ModuleNotFoundError:
        # axon NTFF profiling hook unavailable in this environment
        res = run_bass_kernel_spmd(
            nc, in_maps, core_ids=list(range(NCORES)), trace=False
        )
    return _assemble_loss(res.results, norms64), res


def kernel(queries, items):
    loss, _ = run_on_hw(queries, items)
    return loss

